# revision 29
# baseline (speedup 1.0000x reference)
"""Trainium2 Bass kernel for the quirky-reshape 16-head attention layer.

Shapes (hardcoded): x [2, 2048, 1024], Wq/Wk/Wv/Wo [1024, 1024], n_head=16.

Sharding: core c in [0,8) handles batch b=c//4 and head group g=c%4 (heads
4g..4g+3). The reference's quirky `qkv.reshape(b, s, d)` merge makes output
rows [h*128, (h+1)*128) depend only on head h, so each core produces the
disjoint output row block [g*512, (g+1)*512) of its batch — no collectives.

Precision: q/k path (projections + scores) in fp16, exp / AV / O-projection
in bf16 (fp32 range needed: exp values reach ~1e30), all matmul accumulation
in fp32 PSUM.

Per-core dataflow (transposed-scores streaming attention, ACT-rate paced):
  For each head pair (2 pairs of 2 heads), for each 512-query window (4),
  stream over 16 key blocks kb:
    S^T[kb]  = [kA^T qA | kB^T qB]   two K=64 row-tiled matmuls running
               concurrently in the upper/lower PE array halves (tile_position
               derived from partition ranges) -> one PSUM [128, 1024] fp32
    E[kb]    = exp(S^T[kb])          one ScalarE ACTIVATE over both heads
    AV[h]   += [ones|v_h]^T E[kb,h]  one kb behind the exp pipeline; rows
               0:64 accumulate the softmax denominator, 64:128 the numerator
  Window drain: rcp = reciprocal(denom); broadcast rcp to partitions 64:128
  via DMA; Qs[64:128, hg, q] = qkv * rcp (contiguous bf16); Qs[0:64, hg, q+1]
  = shift-by-one DMA copy of the upper half. The quirky merge then reduces to
  a stride-16 stationary read in the O-projection:
    out_hg = sum_kt Qs[:, hg, (2kt+1)::16]^T Wo[kt]
  QKV projections and O-projections are emitted as single-matmul "filler
  atoms" paced into the attention loop's PE slack (the loop is ACT-bound),
  with deadline-ordered scheduling; input DMAs are chunked and
  priority-ordered so the first projection starts ~4us in.
"""

import numpy as np

B, S, D, H = 2, 2048, 1024, 16
DH = 64
NCORES = 8

_CACHE = {}


def _build_program():
    from concourse import bacc, tile, mybir

    F32 = mybir.dt.float32
    F16 = mybir.dt.float16
    BF16 = mybir.dt.bfloat16
    EXP = mybir.ActivationFunctionType.Exp

    nc = bacc.Bacc(None, target_bir_lowering=False, debug=False)

    xt_d = nc.dram_tensor("xt", [128, 8, 2048], F16, kind="ExternalInput").ap()
    wq_d = nc.dram_tensor("wq", [128, 2, 8, 128], F16, kind="ExternalInput").ap()
    wk_d = nc.dram_tensor("wk", [128, 2, 8, 128], F16, kind="ExternalInput").ap()
    wv_d = nc.dram_tensor("wv", [128, 8, 256], F16, kind="ExternalInput").ap()
    wo_d = nc.dram_tensor("wo", [128, 8, 1024], BF16, kind="ExternalInput").ap()
    out_d = nc.dram_tensor("out", [4, 128, 1024], F32, kind="ExternalOutput").ap()

    with tile.TileContext(nc) as tc:
        with (
            tc.tile_pool(name="keep", bufs=1) as keep,
            tc.tile_pool(name="exp", bufs=6) as expp,
            tc.tile_pool(name="rcp", bufs=2) as rcpp,
            tc.tile_pool(name="osb", bufs=2) as osbp,
            tc.tile_pool(name="ps", bufs=1, space="PSUM") as psp,
        ):
            # ------- persistent SBUF tiles -------
            xtc = [keep.tile([128, 8, 512], F16, tag=f"xtc{c}", name=f"xtc{c}")
                   for c in range(4)]
            wqt = keep.tile([128, 2, 8, 128], F16, tag="wq", name="wqt")
            wkt = keep.tile([128, 2, 8, 128], F16, tag="wk", name="wkt")
            wvt = keep.tile([128, 8, 256], F16, tag="wv", name="wvt")
            wot = keep.tile([128, 8, 1024], BF16, tag="wo", name="wot")
            qT = [keep.tile([128, 2048], F16, tag=f"qT{p}", name=f"qT{p}")
                  for p in range(2)]
            kT = [keep.tile([128, 2048], F16, tag=f"kT{p}", name=f"kT{p}")
                  for p in range(2)]
            vbig = keep.tile([128, 16, 4, 128], BF16, tag="vbig", name="vbig")
            v_sb = [vbig[:, kb] for kb in range(16)]
            qs = keep.tile([128, 4, 2048], BF16, tag="qs", name="qs")

            # ------- input DMAs: chunked, priority-ordered -------
            # scores(0) needs wq + wk + xt chunk 0 (~2.5 MB): wq/wv on
            # scalar, xt c0 split across sync/gpsimd with wk halves right
            # behind, remaining xt chunks and wo streaming after.
            # scalar = fast software-DGE queue (~160GB/s) but each dispatch
            # costs ~1us of ScalarE time -> keep it to 3 dispatches before
            # the ACT-table warm-up; sync/gpsimd hardware queues (~105GB/s
            # each) stream all of xt.
            nc.scalar.dma_start(out=wqt[:, 0], in_=wq_d[:, 0])
            nc.scalar.dma_start(out=wkt[:, 0], in_=wk_d[:, 0])
            nc.scalar.dma_start(out=wvt[:], in_=wv_d[:])
            for c in range(4):
                lo, hi = c * 512, (c + 1) * 512
                nc.sync.dma_start(out=xtc[c][:, 0:4, :], in_=xt_d[:, 0:4, lo:hi])
                nc.gpsimd.dma_start(out=xtc[c][:, 4:8, :],
                                    in_=xt_d[:, 4:8, lo:hi])
            nc.sync.dma_start(out=wqt[:, 1], in_=wq_d[:, 1])
            nc.gpsimd.dma_start(out=wkt[:, 1], in_=wk_d[:, 1])
            nc.sync.dma_start(out=wot[:, 0:4], in_=wo_d[:, 0:4])
            nc.gpsimd.dma_start(out=wot[:, 4:8], in_=wo_d[:, 4:8])

            # ones rows for the denominator trick (disjoint from the V
            # copies); DVE is idle during the initial DMA wait
            nc.vector.memset(vbig[:, 0:4, :, 0:64], 1.0)
            nc.vector.memset(vbig[:, 4:16, :, 0:64], 1.0)

            # pull the ~2.7us exp ACT_TABLE_LOAD off the critical path by
            # issuing a tiny dummy activation before the first real exp,
            # then queue the less-urgent weight DMAs behind it
            warm = keep.tile([1, 8], F32, tag="warm", name="warm")
            nc.vector.memset(warm[:], 0.0)
            nc.scalar.activation(warm[:], warm[:], EXP)

            # ------- matmul-group emit helpers (atom-granular) -------
            def qk_atoms(nm, pair, ch):
                """Project q or k for one 512-token chunk: 8 accumulating
                matmuls (one atom each) + a PSUM->SBUF cast."""
                wt = wqt if nm == "q" else wkt
                dst = qT[pair] if nm == "q" else kT[pair]
                st = {}

                def mk(kt):
                    def f():
                        if kt == 0:
                            st["ps"] = psp.tile([128, 512], F32, tag="fill",
                                                bufs=1, name="qkps")
                        nc.tensor.matmul(
                            st["ps"][:],
                            wt[:, pair, kt, :],
                            xtc[ch][:, kt, :],
                            start=(kt == 0),
                            stop=(kt == 7),
                        )
                        if kt == 7:
                            nc.vector.tensor_copy(
                                dst[:, ch * 512:(ch + 1) * 512], st["ps"][:])
                    return f
                return [(260, mk(kt)) for kt in range(8)]

            def v_atoms(kb):
                """V projection for one 128-token key block: 2 atoms of 4
                matmuls (N=256) + cast into the [ones|v] tile."""
                st = {}

                def mk(half):
                    def f():
                        if half == 0:
                            st["ps"] = psp.tile([128, 512], F32, tag="fill",
                                                bufs=1, name="vps")
                        for kt in range(4 * half, 4 * half + 4):
                            nc.tensor.matmul(
                                st["ps"][:, 0:256],
                                xtc[kb // 4][:, kt, (kb % 4) * 128:(kb % 4 + 1) * 128],
                                wvt[:, kt, :],
                                start=(kt == 0),
                                stop=(kt == 7),
                            )
                        if half == 1:
                            nc.vector.tensor_copy(
                                v_sb[kb][:, :, 64:128],
                                st["ps"][:, 0:256].rearrange(
                                    "p (a b) -> p a b", a=4))
                    return f
                return [(520, mk(0)), (520, mk(1))]

            def oproj_atoms(hg, ps_tag="fill", ps_bufs=1, out_eng=None,
                            halves=(0, 1)):
                """O-projection for head group hg: 2 column halves x 8
                accumulating matmuls with stride-16 stationary reads of qs."""
                atoms = []
                qs_h = qs[:, hg, :].rearrange("p (r t) -> p r t", t=16)
                for h in halves:
                    st = {}

                    def mk(kt, h=h, st=st):
                        def f():
                            if kt == 0:
                                st["ps"] = psp.tile([128, 512], F32, tag=ps_tag,
                                                    bufs=ps_bufs, name="ops")
                            nc.tensor.matmul(
                                st["ps"][:],
                                qs_h[:, :, 2 * kt + 1],
                                wot[:, kt, h * 512:(h + 1) * 512],
                                start=(kt == 0),
                                stop=(kt == 7),
                            )
                            if kt == 7:
                                ot = osbp.tile([128, 512], F32, tag="ot", name="ot")
                                nc.vector.tensor_copy(ot[:], st["ps"][:])
                                (out_eng or nc.sync).dma_start(
                                    out=out_d[hg, :, h * 512:(h + 1) * 512],
                                    in_=ot[:])
                        return f
                    atoms += [(260, mk(kt)) for kt in range(8)]
                return atoms

            # budget-paced filler scheduler (used from window 1 on)
            from collections import deque
            fq = deque()
            bstate = {"b": 0.0}

            def sched_add(gid, atoms):
                for c, fn in atoms:
                    fq.append((c, fn, gid))

            def sched_step(budget=420.0):
                bstate["b"] = min(bstate["b"] + budget, 1300.0)
                while fq and fq[0][0] <= bstate["b"]:
                    c, fn, _ = fq.popleft()
                    bstate["b"] -= c
                    fn()

            def sched_require(*gids):
                """Force-emit queued groups up to and including the given
                gids (compile-order backstop: the PE stream is in-order, so
                prerequisites must be emitted before their consumers)."""
                want = set(gids)
                while want & {g for _, _, g in fq}:
                    c, fn, g = fq.popleft()
                    fn()

            def sched_flush():
                while fq:
                    fq.popleft()[1]()
                bstate["b"] = 0.0

            # ------- flat attention pipeline: 8 windows x 16 kb -------
            # Global iteration g: window w = g//16 (pair w//4, qc w%4),
            # key block kb = g%16. Emitting scores(g) BEFORE AV(g-1) lets
            # the next window's first scores overlap the previous window's
            # last exp (no per-boundary stall). Window drains are emitted
            # one iteration into the next window.
            WINDOWS = [(p, qc) for p in range(2) for qc in range(4)]
            avs = {}      # w -> {hl: av tile}
            ets = {}      # g -> et tile
            scs = {}      # g -> sc tile

            def emit_scores(g):
                w, kb = g // 16, g % 16
                pair, qc = WINDOWS[w]
                q0 = qc * 512
                if kb == 0:
                    avs[w] = {hl: psp.tile([128, 512], F32, tag="av", bufs=3,
                                           name=f"av{hl}")
                              for hl in range(2)}
                sc = psp.tile([128, 1024], F32, tag="sc", bufs=2, name="sc")
                scs[g] = sc
                for hl in range(2):
                    h0 = hl * 64
                    nc.tensor.matmul(
                        sc[:, hl * 512:(hl + 1) * 512],
                        kT[pair][h0:h0 + 64, kb * 128:(kb + 1) * 128],
                        qT[pair][h0:h0 + 64, q0:q0 + 512],
                        start=True,
                        stop=True,
                    )

            def emit_av(g):
                w, kb = g // 16, g % 16
                pair, _ = WINDOWS[w]
                et = ets.pop(g)
                for hl in range(2):
                    nc.tensor.matmul(
                        avs[w][hl][:],
                        v_sb[kb][:, 2 * pair + hl, :],
                        et[:, hl * 512:(hl + 1) * 512],
                        start=(kb == 0),
                        stop=(kb == 15),
                    )

            def emit_exp(g):
                et = expp.tile([128, 1024], BF16, tag="exp", name="et")
                nc.scalar.activation(et[:], scs.pop(g)[:], EXP)
                ets[g] = et

            def emit_drain(w, tail_fill=()):
                """Normalize window w into qs (both heads' chains
                interleaved across engines) + shifted lower copies."""
                pair, qc = WINDOWS[w]
                q0 = qc * 512
                n = 511 if qc == 3 else 512
                av = avs.pop(w)
                rts = {}
                for hl in range(2):
                    rts[hl] = rcpp.tile([128, 512], F32, tag="rcp", name="rt")
                    nc.vector.reciprocal_approx_fast(
                        rts[hl][0:64, :], av[hl][0:64, :])
                    eng = nc.sync if hl == 0 else nc.gpsimd
                    eng.dma_start(out=rts[hl][64:128, :], in_=rts[hl][0:64, :])
                if w == 7:
                    # PE is otherwise idle while the drain chains resolve:
                    # run the rest of the held-back pair-0 O-projection here
                    for _, fn in tail_fill:
                        fn()
                for hl in range(2):
                    hg = 2 * pair + hl
                    nc.vector.tensor_mul(
                        qs[64:128, hg, q0:q0 + 512],
                        av[hl][64:128, :],
                        rts[hl][64:128, :],
                    )
                    eng = nc.gpsimd if hl == 0 else nc.sync
                    eng.dma_start(
                        out=qs[0:64, hg, q0 + 1:q0 + 1 + n],
                        in_=qs[64:128, hg, q0:q0 + n],
                    )
                    if w == 7:
                        for _, fn in oproj_atoms(hg, ps_tag="sc", ps_bufs=2,
                                                 out_eng=nc.scalar):
                            fn()

            # ------- pre-phase: minimum to start window 0 -------
            # dummy matmuls keep the PE busy through the input-DMA wait so
            # the HAM clock gate is already at 8/8 when real work lands
            dums = keep.tile([128, 512], F16, tag="dums", name="dums")
            nc.vector.memset(dums[:], 0.0)
            dps = psp.tile([128, 512], F32, tag="fill", bufs=1, name="dps")
            for _ in range(12):
                nc.tensor.matmul(dps[:], dums[:, 0:128], dums[:],
                                 start=True, stop=True)
            for _, fn in qk_atoms("q", 0, 0):
                fn()
            for _, fn in qk_atoms("k", 0, 0):
                fn()

            # window-0 deadline fillers: v(kb) before iter kb+1, kT chunk c
            # before iter 4c, qT chunk 1 before iter 16 (whole groups only —
            # fill-ring groups must never interleave)
            k01 = qk_atoms("k", 0, 1)
            k02 = qk_atoms("k", 0, 2)
            k03 = qk_atoms("k", 0, 3)
            q01 = qk_atoms("q", 0, 1)
            w0 = {kb: [] for kb in range(16)}
            w0[0] = [v_atoms(0), v_atoms(1), v_atoms(2)]
            w0[1] = [v_atoms(3), v_atoms(4)]
            w0[2] = [v_atoms(5), k01[0:4]]
            w0[3] = [k01[4:8], v_atoms(6)]
            w0[4] = [v_atoms(7), v_atoms(8)]
            w0[5] = [v_atoms(9), k02[0:4]]
            w0[6] = [k02[4:8], v_atoms(10)]
            w0[7] = [v_atoms(11), k03[0:3]]
            w0[8] = [k03[3:6]]
            w0[9] = [k03[6:8], v_atoms(12)]
            w0[10] = [v_atoms(13)]
            w0[11] = [v_atoms(14)]
            w0[12] = [v_atoms(15)]
            w0[13] = [q01[0:4]]
            w0[14] = [q01[4:8]]

            # budget-paced fillers for windows 1+, deadline order
            sched_add("q02", qk_atoms("q", 0, 2))    # by iter 32
            sched_add("k10", qk_atoms("k", 1, 0))    # by iter 64
            sched_add("q03", qk_atoms("q", 0, 3))    # by iter 48
            sched_add("k11", qk_atoms("k", 1, 1))    # by iter 68
            sched_add("k12", qk_atoms("k", 1, 2))    # by iter 72
            sched_add("k13", qk_atoms("k", 1, 3))    # by iter 76
            sched_add("q10", qk_atoms("q", 1, 0))    # by iter 64
            sched_add("q11", qk_atoms("q", 1, 1))    # by iter 80
            sched_add("q12", qk_atoms("q", 1, 2))    # by iter 96
            sched_add("q13", qk_atoms("q", 1, 3))    # by iter 112

            REQUIRE = {32: ("q02",), 48: ("q03",),
                       64: ("k10", "k11", "k12", "k13", "q10"),
                       80: ("q11",), 96: ("q12",), 112: ("q13",)}

            for g in range(128):
                if g in REQUIRE:
                    sched_require(*REQUIRE[g])
                emit_scores(g)
                emit_exp(g)
                # fillers go between scores and AV so PE has work to chew
                # while AV waits on the exp pipeline / drain chains
                if g < 16:
                    for grp in w0.get(g, []):
                        for _, fn in grp:
                            fn()
                else:
                    sched_step()
                if g >= 2:
                    gg = g - 2
                    # previous window's drain must be emitted BEFORE the
                    # first write into the recycled av ring slots (Tile
                    # tracks only already-emitted readers for slot reuse)
                    if gg % 16 == 0 and gg > 0:
                        emit_drain(gg // 16 - 1)
                    emit_av(gg)
                if g == 66:
                    # pair-0 qs complete only after drain(3), emitted at
                    # g=66 under the 2-iteration AV lag
                    sched_add("op0", oproj_atoms(0, halves=(0,)))
            sched_flush()
            emit_av(126)
            tail_fill = (oproj_atoms(0, ps_tag="sc", ps_bufs=2,
                                     out_eng=nc.scalar, halves=(1,))
                         + oproj_atoms(1, ps_tag="sc", ps_bufs=2,
                                       out_eng=nc.scalar))
            for _, fn in tail_fill[0:6]:
                fn()
            emit_av(127)
            emit_drain(7, tail_fill[6:])

    nc.compile()
    return nc


def _get_program():
    if "nc" not in _CACHE:
        _CACHE["nc"] = _build_program()
    return _CACHE["nc"]


def _make_in_maps(x, Wq, Wk, Wv, Wo):
    import ml_dtypes

    bf16 = ml_dtypes.bfloat16
    wo8 = np.ascontiguousarray(
        Wo.astype(bf16).reshape(8, 128, 1024).transpose(1, 0, 2))
    xts = [
        np.ascontiguousarray(
            x[b].T.astype(np.float16).reshape(8, 128, 2048).transpose(1, 0, 2))
        for b in range(B)
    ]
    wq16 = Wq.astype(np.float16)
    wk16 = Wk.astype(np.float16)
    wv16 = Wv.astype(np.float16)
    def pack(w, cols):
        return np.ascontiguousarray(
            w[:, cols].reshape(8, 128, 256).transpose(1, 0, 2))
    def pack_qk(w, cols):
        return np.ascontiguousarray(
            w[:, cols].reshape(8, 128, 2, 128).transpose(1, 2, 0, 3))
    in_maps = []
    for c in range(NCORES):
        b, g = c // 4, c % 4
        cols = slice(4 * g * DH, 4 * (g + 1) * DH)
        in_maps.append(
            {
                "xt": xts[b],
                "wq": pack_qk(wq16, cols),
                "wk": pack_qk(wk16, cols),
                "wv": pack(wv16, cols),
                "wo": wo8,
            }
        )
    return in_maps


def kernel(x, Wq, Wk, Wv, Wo, n_head):
    from concourse.bass_utils import run_bass_kernel_spmd

    assert int(n_head) == H
    x = np.asarray(x, np.float32)
    Wq = np.asarray(Wq, np.float32)
    Wk = np.asarray(Wk, np.float32)
    Wv = np.asarray(Wv, np.float32)
    Wo = np.asarray(Wo, np.float32)

    nc = _get_program()
    in_maps = _make_in_maps(x, Wq, Wk, Wv, Wo)
    res = run_bass_kernel_spmd(nc, in_maps, list(range(NCORES)))

    out = np.empty((B, S, D), np.float32)
    for c in range(NCORES):
        b, g = c // 4, c % 4
        out[b, g * 512:(g + 1) * 512, :] = res.results[c]["out"].reshape(512, 1024)
    return out


# revision 30
# speedup vs baseline: 1.1824x; 1.1824x over previous
"""Trainium2 Bass kernel for the quirky-reshape 16-head attention layer.

Shapes (hardcoded): x [2, 2048, 1024], Wq/Wk/Wv/Wo [1024, 1024], n_head=16.

Sharding: core c in [0,8) handles batch b=c//4 and head group g=c%4 (heads
4g..4g+3). The reference's quirky `qkv.reshape(b, s, d)` merge makes output
rows [h*128, (h+1)*128) depend only on head h, so each core produces the
disjoint output row block [g*512, (g+1)*512) of its batch — no collectives.

Precision: q/k path (projections + scores) in fp16, exp / AV / O-projection
in bf16 (fp32 range needed: exp values reach ~1e30), all matmul accumulation
in fp32 PSUM.

Per-core dataflow (transposed-scores streaming attention, ACT-rate paced):
  For each head pair (2 pairs of 2 heads), for each 512-query window (4),
  stream over 16 key blocks kb:
    S^T[kb]  = [kA^T qA | kB^T qB]   two K=64 row-tiled matmuls running
               concurrently in the upper/lower PE array halves (tile_position
               derived from partition ranges) -> one PSUM [128, 1024] fp32
    E[kb]    = exp(S^T[kb])          one ScalarE ACTIVATE over both heads
    AV[h]   += [ones|v_h]^T E[kb,h]  one kb behind the exp pipeline; rows
               0:64 accumulate the softmax denominator, 64:128 the numerator
  Window drain: rcp = reciprocal(denom); broadcast rcp to partitions 64:128
  via DMA; Qs[64:128, hg, q] = qkv * rcp (contiguous bf16); Qs[0:64, hg, q+1]
  = shift-by-one DMA copy of the upper half. The quirky merge then reduces to
  a stride-16 stationary read in the O-projection:
    out_hg = sum_kt Qs[:, hg, (2kt+1)::16]^T Wo[kt]
  QKV projections and O-projections are emitted as single-matmul "filler
  atoms" paced into the attention loop's PE slack (the loop is ACT-bound),
  with deadline-ordered scheduling; input DMAs are chunked and
  priority-ordered so the first projection starts ~4us in.
"""

import numpy as np

B, S, D, H = 2, 2048, 1024, 16
DH = 64
NCORES = 8

_CACHE = {}


def _build_program():
    from concourse import bacc, tile, mybir

    F32 = mybir.dt.float32
    F16 = mybir.dt.float16
    BF16 = mybir.dt.bfloat16
    EXP = mybir.ActivationFunctionType.Exp

    nc = bacc.Bacc(None, target_bir_lowering=False, debug=False)

    xt_d = nc.dram_tensor("xt", [128, 8, 2048], F16, kind="ExternalInput").ap()
    wq_d = nc.dram_tensor("wq", [128, 2, 8, 128], F16, kind="ExternalInput").ap()
    wk_d = nc.dram_tensor("wk", [128, 2, 8, 128], F16, kind="ExternalInput").ap()
    wv_d = nc.dram_tensor("wv", [128, 8, 256], F16, kind="ExternalInput").ap()
    wo_d = nc.dram_tensor("wo", [128, 8, 1024], BF16, kind="ExternalInput").ap()
    out_d = nc.dram_tensor("out", [4, 128, 1024], F32, kind="ExternalOutput").ap()

    with tile.TileContext(nc) as tc:
        with (
            tc.tile_pool(name="keep", bufs=1) as keep,
            tc.tile_pool(name="exp", bufs=6) as expp,
            tc.tile_pool(name="rcp", bufs=2) as rcpp,
            tc.tile_pool(name="osb", bufs=2) as osbp,
            tc.tile_pool(name="ps", bufs=1, space="PSUM") as psp,
        ):
            # ------- persistent SBUF tiles -------
            xtc = [keep.tile([128, 8, 512], F16, tag=f"xtc{c}", name=f"xtc{c}")
                   for c in range(4)]
            wqt = keep.tile([128, 2, 8, 128], F16, tag="wq", name="wqt")
            wkt = keep.tile([128, 2, 8, 128], F16, tag="wk", name="wkt")
            wvt = keep.tile([128, 8, 256], F16, tag="wv", name="wvt")
            wot = keep.tile([128, 8, 1024], BF16, tag="wo", name="wot")
            qT = [keep.tile([128, 2048], F16, tag=f"qT{p}", name=f"qT{p}")
                  for p in range(2)]
            kT = [keep.tile([128, 2048], F16, tag=f"kT{p}", name=f"kT{p}")
                  for p in range(2)]
            vbig = keep.tile([128, 16, 4, 128], BF16, tag="vbig", name="vbig")
            v_sb = [vbig[:, kb] for kb in range(16)]
            qs = keep.tile([128, 4, 2048], BF16, tag="qs", name="qs")

            # ------- input DMAs: chunked, priority-ordered -------
            # scores(0) needs wq + wk + xt chunk 0 (~2.5 MB): wq/wv on
            # scalar, xt c0 split across sync/gpsimd with wk halves right
            # behind, remaining xt chunks and wo streaming after.
            # scalar = fast software-DGE queue (~160GB/s) but each dispatch
            # costs ~1us of ScalarE time -> keep it to 3 dispatches before
            # the ACT-table warm-up; sync/gpsimd hardware queues (~105GB/s
            # each) stream all of xt.
            nc.scalar.dma_start(out=wqt[:, 0], in_=wq_d[:, 0])
            nc.scalar.dma_start(out=wkt[:, 0], in_=wk_d[:, 0])
            nc.scalar.dma_start(out=wvt[:], in_=wv_d[:])
            for c in range(4):
                lo, hi = c * 512, (c + 1) * 512
                nc.sync.dma_start(out=xtc[c][:, 0:4, :], in_=xt_d[:, 0:4, lo:hi])
                nc.gpsimd.dma_start(out=xtc[c][:, 4:8, :],
                                    in_=xt_d[:, 4:8, lo:hi])
            nc.sync.dma_start(out=wqt[:, 1], in_=wq_d[:, 1])
            nc.gpsimd.dma_start(out=wkt[:, 1], in_=wk_d[:, 1])
            nc.sync.dma_start(out=wot[:, 0:4], in_=wo_d[:, 0:4])
            nc.gpsimd.dma_start(out=wot[:, 4:8], in_=wo_d[:, 4:8])

            # ones rows for the denominator trick (disjoint from the V
            # copies); DVE is idle during the initial DMA wait
            nc.vector.memset(vbig[:, 0:4, :, 0:64], 1.0)
            nc.vector.memset(vbig[:, 4:16, :, 0:64], 1.0)

            # pull the ~2.7us exp ACT_TABLE_LOAD off the critical path by
            # issuing a tiny dummy activation before the first real exp,
            # then queue the less-urgent weight DMAs behind it
            warm = keep.tile([1, 8], F32, tag="warm", name="warm")
            nc.vector.memset(warm[:], 0.0)
            nc.scalar.activation(warm[:], warm[:], EXP)

            # ------- matmul-group emit helpers (atom-granular) -------
            def qk_atoms(nm, pair, ch):
                """Project q or k for one 512-token chunk: 8 accumulating
                matmuls (one atom each) + a PSUM->SBUF cast."""
                wt = wqt if nm == "q" else wkt
                dst = qT[pair] if nm == "q" else kT[pair]
                st = {}

                def mk(kt):
                    def f():
                        if kt == 0:
                            st["ps"] = psp.tile([128, 512], F32, tag="fill",
                                                bufs=1, name="qkps")
                        nc.tensor.matmul(
                            st["ps"][:],
                            wt[:, pair, kt, :],
                            xtc[ch][:, kt, :],
                            start=(kt == 0),
                            stop=(kt == 7),
                        )
                        if kt == 7:
                            nc.vector.tensor_copy(
                                dst[:, ch * 512:(ch + 1) * 512], st["ps"][:])
                    return f
                return [(260, mk(kt)) for kt in range(8)]

            def v_atoms(kb):
                """V projection for one 128-token key block: 2 atoms of 4
                matmuls (N=256) + cast into the [ones|v] tile."""
                st = {}

                def mk(half):
                    def f():
                        if half == 0:
                            st["ps"] = psp.tile([128, 512], F32, tag="fill",
                                                bufs=1, name="vps")
                        for kt in range(4 * half, 4 * half + 4):
                            nc.tensor.matmul(
                                st["ps"][:, 0:256],
                                xtc[kb // 4][:, kt, (kb % 4) * 128:(kb % 4 + 1) * 128],
                                wvt[:, kt, :],
                                start=(kt == 0),
                                stop=(kt == 7),
                            )
                        if half == 1:
                            nc.vector.tensor_copy(
                                v_sb[kb][:, :, 64:128],
                                st["ps"][:, 0:256].rearrange(
                                    "p (a b) -> p a b", a=4))
                    return f
                return [(520, mk(0)), (520, mk(1))]

            def oproj_atoms(hg, ps_tag="fill", ps_bufs=1, out_eng=None,
                            halves=(0, 1)):
                """O-projection for head group hg: 2 column halves x 8
                accumulating matmuls with stride-16 stationary reads of qs."""
                atoms = []
                qs_h = qs[:, hg, :].rearrange("p (r t) -> p r t", t=16)
                for h in halves:
                    st = {}

                    def mk(kt, h=h, st=st):
                        def f():
                            if kt == 0:
                                st["ps"] = psp.tile([128, 512], F32, tag=ps_tag,
                                                    bufs=ps_bufs, name="ops")
                            nc.tensor.matmul(
                                st["ps"][:],
                                qs_h[:, :, 2 * kt + 1],
                                wot[:, kt, h * 512:(h + 1) * 512],
                                start=(kt == 0),
                                stop=(kt == 7),
                            )
                            if kt == 7:
                                ot = osbp.tile([128, 512], F32, tag="ot", name="ot")
                                nc.vector.tensor_copy(ot[:], st["ps"][:])
                                (out_eng or nc.sync).dma_start(
                                    out=out_d[hg, :, h * 512:(h + 1) * 512],
                                    in_=ot[:])
                        return f
                    atoms += [(260, mk(kt)) for kt in range(8)]
                return atoms

            # budget-paced filler scheduler (used from window 1 on)
            from collections import deque
            fq = deque()
            bstate = {"b": 0.0}

            def sched_add(gid, atoms):
                for c, fn in atoms:
                    fq.append((c, fn, gid))

            def sched_step(budget=420.0):
                bstate["b"] = min(bstate["b"] + budget, 1300.0)
                while fq and fq[0][0] <= bstate["b"]:
                    c, fn, _ = fq.popleft()
                    bstate["b"] -= c
                    fn()

            def sched_require(*gids):
                """Force-emit queued groups up to and including the given
                gids (compile-order backstop: the PE stream is in-order, so
                prerequisites must be emitted before their consumers)."""
                want = set(gids)
                while want & {g for _, _, g in fq}:
                    c, fn, g = fq.popleft()
                    fn()

            def sched_flush():
                while fq:
                    fq.popleft()[1]()
                bstate["b"] = 0.0

            # ------- flat attention pipeline: 8 windows x 16 kb -------
            # Global iteration g: window w = g//16 (pair w//4, qc w%4),
            # key block kb = g%16. Emitting scores(g) BEFORE AV(g-1) lets
            # the next window's first scores overlap the previous window's
            # last exp (no per-boundary stall). Window drains are emitted
            # one iteration into the next window.
            WINDOWS = [(p, qc) for p in range(2) for qc in range(4)]
            avs = {}      # w -> {hl: av tile}
            ets = {}      # g -> et tile
            scs = {}      # g -> sc tile

            def emit_scores(g):
                w, kb = g // 16, g % 16
                pair, qc = WINDOWS[w]
                q0 = qc * 512
                if kb == 0:
                    avs[w] = {hl: psp.tile([128, 512], F32, tag="av", bufs=3,
                                           name=f"av{hl}")
                              for hl in range(2)}
                sc = psp.tile([128, 1024], F32, tag="sc", bufs=2, name="sc")
                scs[g] = sc
                for hl in range(2):
                    h0 = hl * 64
                    nc.tensor.matmul(
                        sc[:, hl * 512:(hl + 1) * 512],
                        kT[pair][h0:h0 + 64, kb * 128:(kb + 1) * 128],
                        qT[pair][h0:h0 + 64, q0:q0 + 512],
                        start=True,
                        stop=True,
                    )

            def emit_av(g):
                w, kb = g // 16, g % 16
                pair, _ = WINDOWS[w]
                et = ets.pop(g)
                for hl in range(2):
                    nc.tensor.matmul(
                        avs[w][hl][:],
                        v_sb[kb][:, 2 * pair + hl, :],
                        et[:, hl * 512:(hl + 1) * 512],
                        start=(kb == 0),
                        stop=(kb == 15),
                    )

            def emit_exp(g):
                et = expp.tile([128, 1024], BF16, tag="exp", name="et")
                nc.scalar.activation(et[:], scs.pop(g)[:], EXP)
                ets[g] = et

            def emit_drain(w, tail_fill=()):
                """Normalize window w into qs (both heads' chains
                interleaved across engines) + shifted lower copies."""
                pair, qc = WINDOWS[w]
                q0 = qc * 512
                n = 511 if qc == 3 else 512
                av = avs.pop(w)
                rts = {}
                for hl in range(2):
                    rts[hl] = rcpp.tile([128, 512], F32, tag="rcp", name="rt")
                    nc.vector.reciprocal_approx_fast(
                        rts[hl][0:64, :], av[hl][0:64, :])
                    eng = nc.sync if hl == 0 else nc.gpsimd
                    eng.dma_start(out=rts[hl][64:128, :], in_=rts[hl][0:64, :])
                if w == 7:
                    # PE is otherwise idle while the drain chains resolve:
                    # run the rest of the held-back pair-0 O-projection here
                    for _, fn in tail_fill:
                        fn()
                for hl in range(2):
                    hg = 2 * pair + hl
                    nc.vector.tensor_mul(
                        qs[64:128, hg, q0:q0 + 512],
                        av[hl][64:128, :],
                        rts[hl][64:128, :],
                    )
                    eng = nc.gpsimd if hl == 0 else nc.sync
                    eng.dma_start(
                        out=qs[0:64, hg, q0 + 1:q0 + 1 + n],
                        in_=qs[64:128, hg, q0:q0 + n],
                    )
                    if w == 7:
                        for _, fn in oproj_atoms(hg, ps_tag="sc", ps_bufs=2,
                                                 out_eng=nc.scalar):
                            fn()

            # ------- pre-phase: minimum to start window 0 -------
            # dummy matmuls keep the PE busy through the input-DMA wait so
            # the HAM clock gate is already at 8/8 when real work lands
            dums = keep.tile([128, 512], F16, tag="dums", name="dums")
            nc.vector.memset(dums[:], 0.0)
            dps = psp.tile([128, 512], F32, tag="fill", bufs=1, name="dps")
            for _ in range(12):
                nc.tensor.matmul(dps[:], dums[:, 0:128], dums[:],
                                 start=True, stop=True)
            for _, fn in qk_atoms("q", 0, 0):
                fn()
            for _, fn in qk_atoms("k", 0, 0):
                fn()

            # window-0 deadline fillers: v(kb) before iter kb+1, kT chunk c
            # before iter 4c, qT chunk 1 before iter 16 (whole groups only —
            # fill-ring groups must never interleave)
            k01 = qk_atoms("k", 0, 1)
            k02 = qk_atoms("k", 0, 2)
            k03 = qk_atoms("k", 0, 3)
            q01 = qk_atoms("q", 0, 1)
            w0 = {kb: [] for kb in range(16)}
            w0[0] = [v_atoms(0), v_atoms(1), v_atoms(2)]
            w0[1] = [v_atoms(3), v_atoms(4)]
            w0[2] = [v_atoms(5), k01[0:4]]
            w0[3] = [k01[4:8], v_atoms(6)]
            w0[4] = [v_atoms(7), v_atoms(8)]
            w0[5] = [v_atoms(9), k02[0:4]]
            w0[6] = [k02[4:8], v_atoms(10)]
            w0[7] = [v_atoms(11), k03[0:3]]
            w0[8] = [k03[3:6]]
            w0[9] = [k03[6:8], v_atoms(12)]
            w0[10] = [v_atoms(13)]
            w0[11] = [v_atoms(14)]
            w0[12] = [v_atoms(15)]
            w0[13] = [q01[0:4]]
            w0[14] = [q01[4:8]]

            # budget-paced fillers for windows 1+, deadline order
            sched_add("q02", qk_atoms("q", 0, 2))    # by iter 32
            sched_add("k10", qk_atoms("k", 1, 0))    # by iter 64
            sched_add("q03", qk_atoms("q", 0, 3))    # by iter 48
            sched_add("k11", qk_atoms("k", 1, 1))    # by iter 68
            sched_add("k12", qk_atoms("k", 1, 2))    # by iter 72
            sched_add("k13", qk_atoms("k", 1, 3))    # by iter 76
            sched_add("q10", qk_atoms("q", 1, 0))    # by iter 64
            sched_add("q11", qk_atoms("q", 1, 1))    # by iter 80
            sched_add("q12", qk_atoms("q", 1, 2))    # by iter 96
            sched_add("q13", qk_atoms("q", 1, 3))    # by iter 112

            REQUIRE = {32: ("q02",), 48: ("q03",),
                       64: ("k10", "k11", "k12", "k13", "q10"),
                       80: ("q11",), 96: ("q12",), 112: ("q13",)}

            for g in range(128):
                if g in REQUIRE:
                    sched_require(*REQUIRE[g])
                emit_scores(g)
                emit_exp(g)
                # fillers go between scores and AV so PE has work to chew
                # while AV waits on the exp pipeline / drain chains
                if g < 16:
                    for grp in w0.get(g, []):
                        for _, fn in grp:
                            fn()
                else:
                    sched_step()
                if g >= 2:
                    gg = g - 2
                    # previous window's drain must be emitted BEFORE the
                    # first write into the recycled av ring slots (Tile
                    # tracks only already-emitted readers for slot reuse)
                    if gg % 16 == 0 and gg > 0:
                        emit_drain(gg // 16 - 1)
                    emit_av(gg)
                if g == 66:
                    # pair-0 qs complete only after drain(3), emitted at
                    # g=66 under the 2-iteration AV lag
                    sched_add("op0", oproj_atoms(0, halves=(0,)))
            sched_flush()
            emit_av(126)
            tail_fill = (oproj_atoms(0, ps_tag="sc", ps_bufs=2,
                                     out_eng=nc.scalar, halves=(1,))
                         + oproj_atoms(1, ps_tag="sc", ps_bufs=2,
                                       out_eng=nc.scalar))
            for _, fn in tail_fill[0:8]:
                fn()
            emit_av(127)
            emit_drain(7, tail_fill[8:])

    nc.compile()
    return nc


def _get_program():
    if "nc" not in _CACHE:
        _CACHE["nc"] = _build_program()
    return _CACHE["nc"]


def _make_in_maps(x, Wq, Wk, Wv, Wo):
    import ml_dtypes

    bf16 = ml_dtypes.bfloat16
    wo8 = np.ascontiguousarray(
        Wo.astype(bf16).reshape(8, 128, 1024).transpose(1, 0, 2))
    xts = [
        np.ascontiguousarray(
            x[b].T.astype(np.float16).reshape(8, 128, 2048).transpose(1, 0, 2))
        for b in range(B)
    ]
    wq16 = Wq.astype(np.float16)
    wk16 = Wk.astype(np.float16)
    wv16 = Wv.astype(np.float16)
    def pack(w, cols):
        return np.ascontiguousarray(
            w[:, cols].reshape(8, 128, 256).transpose(1, 0, 2))
    def pack_qk(w, cols):
        return np.ascontiguousarray(
            w[:, cols].reshape(8, 128, 2, 128).transpose(1, 2, 0, 3))
    in_maps = []
    for c in range(NCORES):
        b, g = c // 4, c % 4
        cols = slice(4 * g * DH, 4 * (g + 1) * DH)
        in_maps.append(
            {
                "xt": xts[b],
                "wq": pack_qk(wq16, cols),
                "wk": pack_qk(wk16, cols),
                "wv": pack(wv16, cols),
                "wo": wo8,
            }
        )
    return in_maps


def kernel(x, Wq, Wk, Wv, Wo, n_head):
    from concourse.bass_utils import run_bass_kernel_spmd

    assert int(n_head) == H
    x = np.asarray(x, np.float32)
    Wq = np.asarray(Wq, np.float32)
    Wk = np.asarray(Wk, np.float32)
    Wv = np.asarray(Wv, np.float32)
    Wo = np.asarray(Wo, np.float32)

    nc = _get_program()
    in_maps = _make_in_maps(x, Wq, Wk, Wv, Wo)
    res = run_bass_kernel_spmd(nc, in_maps, list(range(NCORES)))

    out = np.empty((B, S, D), np.float32)
    for c in range(NCORES):
        b, g = c // 4, c % 4
        out[b, g * 512:(g + 1) * 512, :] = res.results[c]["out"].reshape(512, 1024)
    return out


# revision 31
# speedup vs baseline: 1.1967x; 1.0121x over previous
"""Trainium2 Bass kernel for the quirky-reshape 16-head attention layer.

Shapes (hardcoded): x [2, 2048, 1024], Wq/Wk/Wv/Wo [1024, 1024], n_head=16.

Sharding: core c in [0,8) handles batch b=c//4 and head group g=c%4 (heads
4g..4g+3). The reference's quirky `qkv.reshape(b, s, d)` merge makes output
rows [h*128, (h+1)*128) depend only on head h, so each core produces the
disjoint output row block [g*512, (g+1)*512) of its batch — no collectives.

Precision: q/k path (projections + scores) in fp16, exp / AV / O-projection
in bf16 (fp32 range needed: exp values reach ~1e30), all matmul accumulation
in fp32 PSUM.

Per-core dataflow (transposed-scores streaming attention, ACT-rate paced):
  For each head pair (2 pairs of 2 heads), for each 512-query window (4),
  stream over 16 key blocks kb:
    S^T[kb]  = [kA^T qA | kB^T qB]   two K=64 row-tiled matmuls running
               concurrently in the upper/lower PE array halves (tile_position
               derived from partition ranges) -> one PSUM [128, 1024] fp32
    E[kb]    = exp(S^T[kb])          one ScalarE ACTIVATE over both heads
    AV[h]   += [ones|v_h]^T E[kb,h]  one kb behind the exp pipeline; rows
               0:64 accumulate the softmax denominator, 64:128 the numerator
  Window drain: rcp = reciprocal(denom); broadcast rcp to partitions 64:128
  via DMA; Qs[64:128, hg, q] = qkv * rcp (contiguous bf16); Qs[0:64, hg, q+1]
  = shift-by-one DMA copy of the upper half. The quirky merge then reduces to
  a stride-16 stationary read in the O-projection:
    out_hg = sum_kt Qs[:, hg, (2kt+1)::16]^T Wo[kt]
  QKV projections and O-projections are emitted as single-matmul "filler
  atoms" paced into the attention loop's PE slack (the loop is ACT-bound),
  with deadline-ordered scheduling; input DMAs are chunked and
  priority-ordered so the first projection starts ~4us in.
"""

import numpy as np

B, S, D, H = 2, 2048, 1024, 16
DH = 64
NCORES = 8

_CACHE = {}


def _build_program():
    from concourse import bacc, tile, mybir

    F32 = mybir.dt.float32
    F16 = mybir.dt.float16
    BF16 = mybir.dt.bfloat16
    EXP = mybir.ActivationFunctionType.Exp

    nc = bacc.Bacc(None, target_bir_lowering=False, debug=False)

    xt_d = nc.dram_tensor("xt", [128, 8, 2048], F16, kind="ExternalInput").ap()
    wq_d = nc.dram_tensor("wq", [128, 2, 8, 128], F16, kind="ExternalInput").ap()
    wk_d = nc.dram_tensor("wk", [128, 2, 8, 128], F16, kind="ExternalInput").ap()
    wv_d = nc.dram_tensor("wv", [128, 8, 256], F16, kind="ExternalInput").ap()
    wo_d = nc.dram_tensor("wo", [128, 8, 1024], BF16, kind="ExternalInput").ap()
    out_d = nc.dram_tensor("out", [4, 128, 1024], F32, kind="ExternalOutput").ap()

    with tile.TileContext(nc) as tc:
        with (
            tc.tile_pool(name="keep", bufs=1) as keep,
            tc.tile_pool(name="exp", bufs=6) as expp,
            tc.tile_pool(name="rcp", bufs=2) as rcpp,
            tc.tile_pool(name="osb", bufs=2) as osbp,
            tc.tile_pool(name="ps", bufs=1, space="PSUM") as psp,
        ):
            # ------- persistent SBUF tiles -------
            xtc = [keep.tile([128, 8, 512], F16, tag=f"xtc{c}", name=f"xtc{c}")
                   for c in range(4)]
            wqt = keep.tile([128, 2, 8, 128], F16, tag="wq", name="wqt")
            wkt = keep.tile([128, 2, 8, 128], F16, tag="wk", name="wkt")
            wvt = keep.tile([128, 8, 256], F16, tag="wv", name="wvt")
            wot = keep.tile([128, 8, 1024], BF16, tag="wo", name="wot")
            qT = [keep.tile([128, 2048], F16, tag=f"qT{p}", name=f"qT{p}")
                  for p in range(2)]
            kT = [keep.tile([128, 2048], F16, tag=f"kT{p}", name=f"kT{p}")
                  for p in range(2)]
            vbig = keep.tile([128, 16, 4, 128], BF16, tag="vbig", name="vbig")
            v_sb = [vbig[:, kb] for kb in range(16)]
            qs = keep.tile([128, 4, 2048], BF16, tag="qs", name="qs")

            # ------- input DMAs: chunked, priority-ordered -------
            # scores(0) needs wq + wk + xt chunk 0 (~2.5 MB): wq/wv on
            # scalar, xt c0 split across sync/gpsimd with wk halves right
            # behind, remaining xt chunks and wo streaming after.
            # scalar = fast software-DGE queue (~160GB/s) but each dispatch
            # costs ~1us of ScalarE time -> keep it to 3 dispatches before
            # the ACT-table warm-up; sync/gpsimd hardware queues (~105GB/s
            # each) stream all of xt.
            nc.scalar.dma_start(out=wqt[:, 0], in_=wq_d[:, 0])
            nc.scalar.dma_start(out=wkt[:, 0], in_=wk_d[:, 0])
            nc.scalar.dma_start(out=wvt[:], in_=wv_d[:])
            for c in range(4):
                lo, hi = c * 512, (c + 1) * 512
                nc.sync.dma_start(out=xtc[c][:, 0:4, :], in_=xt_d[:, 0:4, lo:hi])
                nc.gpsimd.dma_start(out=xtc[c][:, 4:8, :],
                                    in_=xt_d[:, 4:8, lo:hi])
            nc.sync.dma_start(out=wqt[:, 1], in_=wq_d[:, 1])
            nc.gpsimd.dma_start(out=wkt[:, 1], in_=wk_d[:, 1])
            nc.sync.dma_start(out=wot[:, 0:4], in_=wo_d[:, 0:4])
            nc.gpsimd.dma_start(out=wot[:, 4:8], in_=wo_d[:, 4:8])

            # ones rows for the denominator trick (disjoint from the V
            # copies); DVE is idle during the initial DMA wait
            nc.vector.memset(vbig[:, 0:4, :, 0:64], 1.0)
            nc.vector.memset(vbig[:, 4:16, :, 0:64], 1.0)

            # pull the ~2.7us exp ACT_TABLE_LOAD off the critical path by
            # issuing a tiny dummy activation before the first real exp,
            # then queue the less-urgent weight DMAs behind it
            warm = keep.tile([1, 8], F32, tag="warm", name="warm")
            nc.vector.memset(warm[:], 0.0)
            nc.scalar.activation(warm[:], warm[:], EXP)

            # ------- matmul-group emit helpers (atom-granular) -------
            def qk_atoms(nm, pair, ch):
                """Project q or k for one 512-token chunk: 8 accumulating
                matmuls (one atom each) + a PSUM->SBUF cast."""
                wt = wqt if nm == "q" else wkt
                dst = qT[pair] if nm == "q" else kT[pair]
                st = {}

                def mk(kt):
                    def f():
                        if kt == 0:
                            st["ps"] = psp.tile([128, 512], F32, tag="fill",
                                                bufs=1, name="qkps")
                        nc.tensor.matmul(
                            st["ps"][:],
                            wt[:, pair, kt, :],
                            xtc[ch][:, kt, :],
                            start=(kt == 0),
                            stop=(kt == 7),
                        )
                        if kt == 7:
                            nc.vector.tensor_copy(
                                dst[:, ch * 512:(ch + 1) * 512], st["ps"][:])
                    return f
                return [(260, mk(kt)) for kt in range(8)]

            def v_atoms(kb):
                """V projection for one 128-token key block: 2 atoms of 4
                matmuls (N=256) + cast into the [ones|v] tile."""
                st = {}

                def mk(half):
                    def f():
                        if half == 0:
                            st["ps"] = psp.tile([128, 512], F32, tag="fill",
                                                bufs=1, name="vps")
                        for kt in range(4 * half, 4 * half + 4):
                            nc.tensor.matmul(
                                st["ps"][:, 0:256],
                                xtc[kb // 4][:, kt, (kb % 4) * 128:(kb % 4 + 1) * 128],
                                wvt[:, kt, :],
                                start=(kt == 0),
                                stop=(kt == 7),
                            )
                        if half == 1:
                            nc.vector.tensor_copy(
                                v_sb[kb][:, :, 64:128],
                                st["ps"][:, 0:256].rearrange(
                                    "p (a b) -> p a b", a=4))
                    return f
                return [(520, mk(0)), (520, mk(1))]

            def oproj_atoms(hg, ps_tag="fill", ps_bufs=1, out_eng=None,
                            halves=(0, 1)):
                """O-projection for head group hg: 2 column halves x 8
                accumulating matmuls with stride-16 stationary reads of qs."""
                atoms = []
                qs_h = qs[:, hg, :].rearrange("p (r t) -> p r t", t=16)
                for h in halves:
                    st = {}

                    def mk(kt, h=h, st=st):
                        def f():
                            if kt == 0:
                                st["ps"] = psp.tile([128, 512], F32, tag=ps_tag,
                                                    bufs=ps_bufs, name="ops")
                            nc.tensor.matmul(
                                st["ps"][:],
                                qs_h[:, :, 2 * kt + 1],
                                wot[:, kt, h * 512:(h + 1) * 512],
                                start=(kt == 0),
                                stop=(kt == 7),
                            )
                            if kt == 7:
                                ot = osbp.tile([128, 512], F32, tag="ot", name="ot")
                                nc.vector.tensor_copy(ot[:], st["ps"][:])
                                (out_eng or nc.sync).dma_start(
                                    out=out_d[hg, :, h * 512:(h + 1) * 512],
                                    in_=ot[:])
                        return f
                    atoms += [(260, mk(kt)) for kt in range(8)]
                return atoms

            # budget-paced filler scheduler (used from window 1 on)
            from collections import deque
            fq = deque()
            bstate = {"b": 0.0}

            def sched_add(gid, atoms):
                for c, fn in atoms:
                    fq.append((c, fn, gid))

            def sched_step(budget=420.0):
                bstate["b"] = min(bstate["b"] + budget, 1300.0)
                while fq and fq[0][0] <= bstate["b"]:
                    c, fn, _ = fq.popleft()
                    bstate["b"] -= c
                    fn()

            def sched_require(*gids):
                """Force-emit queued groups up to and including the given
                gids (compile-order backstop: the PE stream is in-order, so
                prerequisites must be emitted before their consumers)."""
                want = set(gids)
                while want & {g for _, _, g in fq}:
                    c, fn, g = fq.popleft()
                    fn()

            def sched_flush():
                while fq:
                    fq.popleft()[1]()
                bstate["b"] = 0.0

            # ------- flat attention pipeline: 8 windows x 16 kb -------
            # Global iteration g: window w = g//16 (pair w//4, qc w%4),
            # key block kb = g%16. Emitting scores(g) BEFORE AV(g-1) lets
            # the next window's first scores overlap the previous window's
            # last exp (no per-boundary stall). Window drains are emitted
            # one iteration into the next window.
            WINDOWS = [(p, qc) for p in range(2) for qc in range(4)]
            avs = {}      # w -> {hl: av tile}
            ets = {}      # g -> et tile
            scs = {}      # g -> sc tile

            def emit_scores(g):
                w, kb = g // 16, g % 16
                pair, qc = WINDOWS[w]
                q0 = qc * 512
                if kb == 0:
                    avs[w] = {hl: psp.tile([128, 512], F32, tag="av", bufs=3,
                                           name=f"av{hl}")
                              for hl in range(2)}
                sc = psp.tile([128, 1024], F32, tag="sc", bufs=2, name="sc")
                scs[g] = sc
                for hl in range(2):
                    h0 = hl * 64
                    nc.tensor.matmul(
                        sc[:, hl * 512:(hl + 1) * 512],
                        kT[pair][h0:h0 + 64, kb * 128:(kb + 1) * 128],
                        qT[pair][h0:h0 + 64, q0:q0 + 512],
                        start=True,
                        stop=True,
                    )

            def emit_av(g):
                w, kb = g // 16, g % 16
                pair, _ = WINDOWS[w]
                et = ets.pop(g)
                for hl in range(2):
                    nc.tensor.matmul(
                        avs[w][hl][:],
                        v_sb[kb][:, 2 * pair + hl, :],
                        et[:, hl * 512:(hl + 1) * 512],
                        start=(kb == 0),
                        stop=(kb == 15),
                    )

            def emit_exp(g):
                et = expp.tile([128, 1024], BF16, tag="exp", name="et")
                nc.scalar.activation(et[:], scs.pop(g)[:], EXP)
                ets[g] = et

            def emit_drain(w, tail_fill=()):
                """Normalize window w into qs (both heads' chains
                interleaved across engines) + shifted lower copies."""
                pair, qc = WINDOWS[w]
                q0 = qc * 512
                n = 511 if qc == 3 else 512
                av = avs.pop(w)
                rts = {}
                for hl in range(2):
                    rts[hl] = rcpp.tile([128, 512], F32, tag="rcp", name="rt")
                    nc.vector.reciprocal_approx_fast(
                        rts[hl][0:64, :], av[hl][0:64, :])
                    eng = nc.sync if hl == 0 else nc.gpsimd
                    eng.dma_start(out=rts[hl][64:128, :], in_=rts[hl][0:64, :])
                if w == 7:
                    # PE is otherwise idle while the drain chains resolve:
                    # run the rest of the held-back pair-0 O-projection here
                    for _, fn in tail_fill:
                        fn()
                for hl in range(2):
                    hg = 2 * pair + hl
                    nc.vector.tensor_mul(
                        qs[64:128, hg, q0:q0 + 512],
                        av[hl][64:128, :],
                        rts[hl][64:128, :],
                    )
                    eng = nc.gpsimd if hl == 0 else nc.sync
                    eng.dma_start(
                        out=qs[0:64, hg, q0 + 1:q0 + 1 + n],
                        in_=qs[64:128, hg, q0:q0 + n],
                    )
                    if w == 7:
                        for _, fn in oproj_atoms(hg, ps_tag="sc", ps_bufs=2,
                                                 out_eng=nc.scalar):
                            fn()

            # ------- pre-phase: minimum to start window 0 -------
            # dummy matmuls keep the PE busy through the input-DMA wait so
            # the HAM clock gate is already at 8/8 when real work lands
            dums = keep.tile([128, 512], F16, tag="dums", name="dums")
            nc.vector.memset(dums[:], 0.0)
            dps = psp.tile([128, 512], F32, tag="fill", bufs=1, name="dps")
            for _ in range(12):
                nc.tensor.matmul(dps[:], dums[:, 0:128], dums[:],
                                 start=True, stop=True)
            for _, fn in qk_atoms("q", 0, 0):
                fn()
            for _, fn in qk_atoms("k", 0, 0):
                fn()

            # window-0 deadline fillers: v(kb) before iter kb+1, kT chunk c
            # before iter 4c, qT chunk 1 before iter 16 (whole groups only —
            # fill-ring groups must never interleave)
            k01 = qk_atoms("k", 0, 1)
            k02 = qk_atoms("k", 0, 2)
            k03 = qk_atoms("k", 0, 3)
            q01 = qk_atoms("q", 0, 1)
            w0 = {kb: [] for kb in range(16)}
            w0[0] = [v_atoms(0), v_atoms(1), v_atoms(2)]
            w0[1] = [v_atoms(3), v_atoms(4)]
            w0[2] = [v_atoms(5), k01[0:4]]
            w0[3] = [k01[4:8], v_atoms(6)]
            w0[4] = [v_atoms(7), v_atoms(8)]
            w0[5] = [v_atoms(9), k02[0:4]]
            w0[6] = [k02[4:8], v_atoms(10)]
            w0[7] = [v_atoms(11), k03[0:3]]
            w0[8] = [k03[3:6]]
            w0[9] = [k03[6:8], v_atoms(12)]
            w0[10] = [v_atoms(13)]
            w0[11] = [v_atoms(14)]
            w0[12] = [v_atoms(15)]
            w0[13] = [q01[0:4]]
            w0[14] = [q01[4:8]]

            # budget-paced fillers for windows 1+, deadline order
            sched_add("q02", qk_atoms("q", 0, 2))    # by iter 32
            sched_add("k10", qk_atoms("k", 1, 0))    # by iter 64
            sched_add("q03", qk_atoms("q", 0, 3))    # by iter 48
            sched_add("k11", qk_atoms("k", 1, 1))    # by iter 68
            sched_add("k12", qk_atoms("k", 1, 2))    # by iter 72
            sched_add("k13", qk_atoms("k", 1, 3))    # by iter 76
            sched_add("q10", qk_atoms("q", 1, 0))    # by iter 64
            sched_add("q11", qk_atoms("q", 1, 1))    # by iter 80
            sched_add("q12", qk_atoms("q", 1, 2))    # by iter 96
            sched_add("q13", qk_atoms("q", 1, 3))    # by iter 112

            REQUIRE = {32: ("q02",), 48: ("q03",),
                       64: ("k10", "k11", "k12", "k13", "q10"),
                       80: ("q11",), 96: ("q12",), 112: ("q13",)}

            for g in range(128):
                if g in REQUIRE:
                    sched_require(*REQUIRE[g])
                emit_scores(g)
                emit_exp(g)
                # fillers go between scores and AV so PE has work to chew
                # while AV waits on the exp pipeline / drain chains
                if g < 16:
                    for grp in w0.get(g, []):
                        for _, fn in grp:
                            fn()
                else:
                    sched_step()
                if g >= 2:
                    gg = g - 2
                    # previous window's drain must be emitted BEFORE the
                    # first write into the recycled av ring slots (Tile
                    # tracks only already-emitted readers for slot reuse)
                    if gg % 16 == 0 and gg > 0:
                        emit_drain(gg // 16 - 1)
                    emit_av(gg)
                if g == 66:
                    # pair-0 qs complete only after drain(3), emitted at
                    # g=66 under the 2-iteration AV lag
                    sched_add("op0", oproj_atoms(0, halves=(0,)))
            sched_flush()
            emit_av(126)
            tail_fill = (oproj_atoms(0, ps_tag="sc", ps_bufs=2,
                                     out_eng=nc.scalar, halves=(1,))
                         + oproj_atoms(1, ps_tag="sc", ps_bufs=2,
                                       out_eng=nc.scalar))
            for _, fn in tail_fill[0:6]:
                fn()
            emit_av(127)
            emit_drain(7, tail_fill[6:])

    nc.compile()
    return nc


def _get_program():
    if "nc" not in _CACHE:
        _CACHE["nc"] = _build_program()
    return _CACHE["nc"]


def _make_in_maps(x, Wq, Wk, Wv, Wo):
    import ml_dtypes

    bf16 = ml_dtypes.bfloat16
    wo8 = np.ascontiguousarray(
        Wo.astype(bf16).reshape(8, 128, 1024).transpose(1, 0, 2))
    xts = [
        np.ascontiguousarray(
            x[b].T.astype(np.float16).reshape(8, 128, 2048).transpose(1, 0, 2))
        for b in range(B)
    ]
    wq16 = Wq.astype(np.float16)
    wk16 = Wk.astype(np.float16)
    wv16 = Wv.astype(np.float16)
    def pack(w, cols):
        return np.ascontiguousarray(
            w[:, cols].reshape(8, 128, 256).transpose(1, 0, 2))
    def pack_qk(w, cols):
        return np.ascontiguousarray(
            w[:, cols].reshape(8, 128, 2, 128).transpose(1, 2, 0, 3))
    in_maps = []
    for c in range(NCORES):
        b, g = c // 4, c % 4
        cols = slice(4 * g * DH, 4 * (g + 1) * DH)
        in_maps.append(
            {
                "xt": xts[b],
                "wq": pack_qk(wq16, cols),
                "wk": pack_qk(wk16, cols),
                "wv": pack(wv16, cols),
                "wo": wo8,
            }
        )
    return in_maps


def kernel(x, Wq, Wk, Wv, Wo, n_head):
    from concourse.bass_utils import run_bass_kernel_spmd

    assert int(n_head) == H
    x = np.asarray(x, np.float32)
    Wq = np.asarray(Wq, np.float32)
    Wk = np.asarray(Wk, np.float32)
    Wv = np.asarray(Wv, np.float32)
    Wo = np.asarray(Wo, np.float32)

    nc = _get_program()
    in_maps = _make_in_maps(x, Wq, Wk, Wv, Wo)
    res = run_bass_kernel_spmd(nc, in_maps, list(range(NCORES)))

    out = np.empty((B, S, D), np.float32)
    for c in range(NCORES):
        b, g = c // 4, c % 4
        out[b, g * 512:(g + 1) * 512, :] = res.results[c]["out"].reshape(512, 1024)
    return out
